# revision 19
# baseline (speedup 1.0000x reference)
"""Conv2D 3x3 stride-1 pad-1 (NCHW) as implicit GEMM on 8 NeuronCores.

Strategy: data-parallel over batch (32 imgs -> 4 per core). The input is
zero-padded on the host to (*, 128, 58, 58) so each image DMAs contiguously
into an SBUF tile [C=128, 58, 58] with input channels on partitions.
Weights are preprocessed host-side to [I=128, (kh kw o)] so each
(tap, ochunk) slice is a ready [K=128, M=128] stationary operand.
Output rows are processed in groups of 8 (moving free dim N = 8*56 = 448),
accumulating the 9 filter taps into one PSUM bank per row-group.

x (4,128,58,58) -> out (4,256,56,56) f32 per core; no collectives.
"""

import os
import sys

import numpy as np

if "/opt/trn_rl_repo" not in sys.path:
    sys.path.insert(0, "/opt/trn_rl_repo")

from concourse import bacc, bass, mybir  # noqa: E402
from concourse.bass_utils import run_bass_kernel_spmd  # noqa: E402
from concourse.tile import TileContext  # noqa: E402

N_FULL, CIN, H, W = 32, 128, 56, 56
COUT = 256
KH = KW = 3
NCORES = 8
NPER = N_FULL // NCORES  # 4 images per core
HP, WP = H + 2, W + 2  # 58 x 58 padded
ROWS = 8  # output rows per matmul group
NFREE = ROWS * W  # 448 moving free dim (<= 512 for 4-byte dtypes)
NGROUPS = H // ROWS  # 7
OCH = COUT // 128  # 2 output-channel chunks

# fp32r: full fp32 data streamed through the PE at bf16 rate (free dim >= 256).
MODE = os.environ.get("CONV_MM_MODE", "fp32r")

_CACHE = {}


def _build_conv(mode):
    f32 = mybir.dt.float32
    bf16 = mybir.dt.bfloat16
    if mode == "fp32":
        mm_dt, io_dt = f32, f32
    elif mode == "fp32r":
        mm_dt, io_dt = mybir.dt.float32r, f32
    elif mode in ("bf16", "split3"):
        mm_dt, io_dt = bf16, bf16
    else:
        raise ValueError(mode)

    # Bacc (not raw Bass): its compile pipeline legalizes sync waits --
    # TRN2 instructions carry at most one wait slot.
    nc = bacc.Bacc(None, target_bir_lowering=False)

    if mode == "split3":
        x_names = ["xh", "xl"]
        w_names = ["wh", "wl"]
        # (x_idx, w_idx) matmul passes: hh + hl + lh ~ full fp32 product
        terms = [(0, 0), (0, 1), (1, 0)]
    else:
        x_names = ["x"]
        w_names = ["wt"]
        terms = [(0, 0)]

    x_par = [
        nc.declare_dram_parameter(nm, [NPER, CIN, HP, WP], io_dt, isOutput=False)
        for nm in x_names
    ]
    w_par = [
        nc.declare_dram_parameter(nm, [CIN, KH * KW * COUT], io_dt, isOutput=False)
        for nm in w_names
    ]
    bias_par = nc.declare_dram_parameter("bias", [COUT], f32, isOutput=False)
    out_par = nc.declare_dram_parameter("out", [NPER, COUT, H, W], f32, isOutput=True)
    out_flat = out_par.rearrange("n o h w -> n o (h w)")

    def mmview(ap):
        return ap.bitcast(mm_dt) if mm_dt != io_dt else ap

    nmm_per_psum = KH * KW * len(terms)

    with TileContext(nc) as tc:
        with (
            tc.tile_pool(name="const", bufs=1) as cpool,
            tc.tile_pool(name="xpad", bufs=1) as xpool,
            tc.tile_pool(name="psum", bufs=8, space="PSUM") as ppool,
            tc.tile_pool(name="outp", bufs=4) as opool,
        ):
            w_sb = []
            for wi, wp in enumerate(w_par):
                t = cpool.tile([CIN, KH * KW * COUT], mm_dt, tag=f"w{wi}", name="w")
                nc.sync.dma_start(out=t[:], in_=mmview(wp[:]))
                w_sb.append(t)
            bias_sb = cpool.tile([128, OCH], f32, tag="bias")
            nc.sync.dma_start(
                out=bias_sb[:], in_=bias_par.rearrange("(a b) -> b a", b=128)
            )

            # Two padded-x buffers per input tensor (double buffering across
            # images); the zero borders come in with the host-padded DMA.
            xpads = []  # [buf][x_idx] -> tile
            for b in range(2):
                per_buf = []
                for xi in range(len(x_par)):
                    t = xpool.tile(
                        [CIN, HP, WP], mm_dt, tag=f"xpad{b}_{xi}", name="xpad"
                    )
                    per_buf.append(t)
                xpads.append(per_buf)

            for n in range(NPER):
                bufs = xpads[n % 2]
                for xi, xp in enumerate(x_par):
                    nc.sync.dma_start(out=bufs[xi][:], in_=mmview(xp[n]))
                for oc in range(OCH):
                    psums = [
                        ppool.tile([128, NFREE], f32, tag="ps", name="ps")
                        for _ in range(NGROUPS)
                    ]
                    i_mm = 0
                    for xi, wi in terms:
                        xt = bufs[xi]
                        for tap in range(KH * KW):
                            kh, kw = divmod(tap, KW)
                            lhsT = w_sb[wi][
                                :, tap * COUT + oc * 128 : tap * COUT + oc * 128 + 128
                            ]
                            for g in range(NGROUPS):
                                nc.tensor.matmul(
                                    psums[g][:],
                                    lhsT,
                                    xt[
                                        :,
                                        g * ROWS + kh : g * ROWS + kh + ROWS,
                                        kw : kw + W,
                                    ],
                                    start=(i_mm == 0),
                                    stop=(i_mm == nmm_per_psum - 1),
                                )
                            i_mm += 1
                    for g in range(NGROUPS):
                        ot = opool.tile([128, NFREE], f32, tag="ot", name="ot")
                        nc.vector.tensor_scalar_add(
                            out=ot[:], in0=psums[g][:], scalar1=bias_sb[:, oc : oc + 1]
                        )
                        nc.sync.dma_start(
                            out=out_flat[
                                n,
                                oc * 128 : (oc + 1) * 128,
                                g * NFREE : (g + 1) * NFREE,
                            ],
                            in_=ot[:],
                        )
    nc.compile()
    return nc


def _get_nc(mode):
    if mode not in _CACHE:
        _CACHE[mode] = _build_conv(mode)
    return _CACHE[mode]


# test-harness hooks: set TRACE=True before calling kernel() to capture an
# NTFF profile; LAST_RESULTS then holds the BassKernelResults.
TRACE = False
LAST_RESULTS = None


def kernel(x, weight, bias):
    global LAST_RESULTS
    mode = MODE
    x = np.ascontiguousarray(np.asarray(x), dtype=np.float32)
    w = np.ascontiguousarray(np.asarray(weight), dtype=np.float32)
    b = np.ascontiguousarray(np.asarray(bias), dtype=np.float32)
    xp = np.pad(x, ((0, 0), (0, 0), (1, 1), (1, 1)))
    # wt[i, (kh kw o)] = w[o, i, kh, kw]
    wt = np.ascontiguousarray(w.transpose(1, 2, 3, 0).reshape(CIN, KH * KW * COUT))

    if mode in ("fp32", "fp32r"):
        per_core = [
            {"x": xp[c * NPER : (c + 1) * NPER], "wt": wt, "bias": b}
            for c in range(NCORES)
        ]
    else:
        import ml_dtypes

        bfl = ml_dtypes.bfloat16
        if mode == "bf16":
            xh = xp.astype(bfl)
            wth = wt.astype(bfl)
            per_core = [
                {"x": xh[c * NPER : (c + 1) * NPER], "wt": wth, "bias": b}
                for c in range(NCORES)
            ]
        else:  # split3
            xh = xp.astype(bfl)
            xl = (xp - xh.astype(np.float32)).astype(bfl)
            wh = wt.astype(bfl)
            wl = (wt - wh.astype(np.float32)).astype(bfl)
            per_core = [
                {
                    "xh": xh[c * NPER : (c + 1) * NPER],
                    "xl": xl[c * NPER : (c + 1) * NPER],
                    "wh": wh,
                    "wl": wl,
                    "bias": b,
                }
                for c in range(NCORES)
            ]

    kwargs = {}
    if TRACE:
        kwargs = dict(trace=True, trace_cores=[0])
    res = run_bass_kernel_spmd(
        _get_nc(mode), per_core, core_ids=list(range(NCORES)), **kwargs
    )
    LAST_RESULTS = res
    return np.concatenate([r["out"] for r in res.results], axis=0)


# revision 24
# speedup vs baseline: 1.0353x; 1.0353x over previous
"""Conv2D 3x3 stride-1 pad-1 (NCHW) as implicit GEMM on 8 NeuronCores.

Strategy: data-parallel over batch (32 imgs -> 4 per core). The input is
zero-padded on the host to (*, 128, 58, 58) so each image DMAs contiguously
into an SBUF tile [C=128, 58, 58] with input channels on partitions.
Weights are preprocessed host-side to [I=128, (kh kw o)] so each
(tap, ochunk) slice is a ready [K=128, M=128] stationary operand.
Output rows are processed in groups of 8 (moving free dim N = 8*56 = 448),
accumulating the 9 filter taps into one PSUM bank per row-group.

x (4,128,58,58) -> out (4,256,56,56) f32 per core; no collectives.
"""

import os
import sys

import numpy as np

if "/opt/trn_rl_repo" not in sys.path:
    sys.path.insert(0, "/opt/trn_rl_repo")

from concourse import bacc, bass, mybir  # noqa: E402
from concourse.bass_utils import run_bass_kernel_spmd  # noqa: E402
from concourse.tile import TileContext, add_dep_helper  # noqa: E402

N_FULL, CIN, H, W = 32, 128, 56, 56
COUT = 256
KH = KW = 3
NCORES = 8
NPER = N_FULL // NCORES  # 4 images per core
HP, WP = H + 2, W + 2  # 58 x 58 padded
ROWS = 8  # output rows per matmul group
NFREE = ROWS * W  # 448 moving free dim (<= 512 for 4-byte dtypes)
NGROUPS = H // ROWS  # 7
OCH = COUT // 128  # 2 output-channel chunks

# fp32r: full fp32 data streamed through the PE at bf16 rate (free dim >= 256).
MODE = os.environ.get("CONV_MM_MODE", "fp32r")

_CACHE = {}


def _build_conv(mode):
    f32 = mybir.dt.float32
    bf16 = mybir.dt.bfloat16
    if mode == "fp32":
        mm_dt, io_dt = f32, f32
    elif mode == "fp32r":
        mm_dt, io_dt = mybir.dt.float32r, f32
    elif mode in ("bf16", "split3"):
        mm_dt, io_dt = bf16, bf16
    else:
        raise ValueError(mode)

    # Bacc (not raw Bass): its compile pipeline legalizes sync waits --
    # TRN2 instructions carry at most one wait slot.
    nc = bacc.Bacc(None, target_bir_lowering=False)

    if mode == "split3":
        x_names = ["xh", "xl"]
        w_names = ["wh", "wl"]
        # (x_idx, w_idx) matmul passes: hh + hl + lh ~ full fp32 product
        terms = [(0, 0), (0, 1), (1, 0)]
    else:
        x_names = ["x"]
        w_names = ["wt"]
        terms = [(0, 0)]

    x_par = [
        nc.declare_dram_parameter(nm, [NPER, CIN, HP, WP], io_dt, isOutput=False)
        for nm in x_names
    ]
    w_par = [
        nc.declare_dram_parameter(nm, [CIN, KH * KW * COUT], io_dt, isOutput=False)
        for nm in w_names
    ]
    bias_par = nc.declare_dram_parameter("bias", [COUT], f32, isOutput=False)
    out_par = nc.declare_dram_parameter("out", [NPER, COUT, H, W], f32, isOutput=True)
    out_flat = out_par.rearrange("n o h w -> n o (h w)")

    def mmview(ap):
        return ap.bitcast(mm_dt) if mm_dt != io_dt else ap

    nmm_per_psum = KH * KW * len(terms)

    with TileContext(nc) as tc:
        with (
            tc.tile_pool(name="const", bufs=1) as cpool,
            tc.tile_pool(name="xpad", bufs=1) as xpool,
            tc.tile_pool(name="psum", bufs=8, space="PSUM") as ppool,
            tc.tile_pool(name="outp", bufs=4) as opool,
        ):
            # Weights arrive in two chunks: the oc0 halves of all taps are on
            # the critical path to the first matmul; oc1 halves are deferred
            # behind it so the first-image load gets full HBM bandwidth.
            w_sb = []
            tail_dmas = []  # released once the first matmul has issued
            for wi, wp in enumerate(w_par):
                t = cpool.tile([CIN, KH * KW * COUT], mm_dt, tag=f"w{wi}", name="w")
                t3 = t.rearrange("p (t o) -> p t o", t=KH * KW)
                w3 = mmview(wp[:]).rearrange("p (t o) -> p t o", t=KH * KW)
                nc.sync.dma_start(out=t3[:, :, 0:128], in_=w3[:, :, 0:128])
                d = nc.sync.dma_start(out=t3[:, :, 128:256], in_=w3[:, :, 128:256])
                tail_dmas.append(d)
                w_sb.append(t)
            bias_sb = cpool.tile([128, OCH], f32, tag="bias")
            nc.sync.dma_start(
                out=bias_sb[:], in_=bias_par.rearrange("(a b) -> b a", b=128)
            )

            # Two padded-x buffers per input tensor (double buffering across
            # images); the zero borders come in with the host-padded DMA.
            xpads = []  # [buf][x_idx] -> tile
            for b in range(2):
                per_buf = []
                for xi in range(len(x_par)):
                    t = xpool.tile(
                        [CIN, HP, WP], mm_dt, tag=f"xpad{b}_{xi}", name="xpad"
                    )
                    per_buf.append(t)
                xpads.append(per_buf)

            XSPLIT = 34  # padded rows [0,34) cover row-groups 0-3
            mm_first = None
            mm_oc1_first = None
            x1_dmas = []  # image-1 loads, deferred until the oc1 pass starts
            for n in range(NPER):
                bufs = xpads[n % 2]
                for xi, xp in enumerate(x_par):
                    if n == 0:
                        # Two row chunks: groups 0-3 can start after chunk A.
                        nc.sync.dma_start(
                            out=bufs[xi][:, 0:XSPLIT, :],
                            in_=mmview(xp[0])[:, 0:XSPLIT, :],
                        )
                        d = nc.sync.dma_start(
                            out=bufs[xi][:, XSPLIT:HP, :],
                            in_=mmview(xp[0])[:, XSPLIT:HP, :],
                        )
                        tail_dmas.append(d)
                    else:
                        # gpsimd queue: slot-reuse waits must not block the
                        # sync queue's output DMAs.
                        d = nc.gpsimd.dma_start(out=bufs[xi][:], in_=mmview(xp[n]))
                        if n == 1:
                            x1_dmas.append(d)
                for oc in range(OCH):
                    psums = [
                        ppool.tile([128, NFREE], f32, tag="ps", name="ps")
                        for _ in range(NGROUPS)
                    ]
                    i_mm = 0
                    for xi, wi in terms:
                        xt = bufs[xi]
                        for tap in range(KH * KW):
                            kh, kw = divmod(tap, KW)
                            lhsT = w_sb[wi][
                                :, tap * COUT + oc * 128 : tap * COUT + oc * 128 + 128
                            ]
                            for g in range(NGROUPS):
                                mm = nc.tensor.matmul(
                                    psums[g][:],
                                    lhsT,
                                    xt[
                                        :,
                                        g * ROWS + kh : g * ROWS + kh + ROWS,
                                        kw : kw + W,
                                    ],
                                    start=(i_mm == 0),
                                    stop=(i_mm == nmm_per_psum - 1),
                                )
                                if n == 0 and i_mm == 0 and g == 0:
                                    if oc == 0:
                                        mm_first = mm
                                    else:
                                        mm_oc1_first = mm
                            i_mm += 1
                    for g in range(NGROUPS):
                        ot = opool.tile([128, NFREE], f32, tag="ot", name="ot")
                        nc.vector.tensor_scalar_add(
                            out=ot[:], in0=psums[g][:], scalar1=bias_sb[:, oc : oc + 1]
                        )
                        nc.sync.dma_start(
                            out=out_flat[
                                n,
                                oc * 128 : (oc + 1) * 128,
                                g * NFREE : (g + 1) * NFREE,
                            ],
                            in_=ot[:],
                        )
            for d in tail_dmas:
                add_dep_helper(
                    d.ins, mm_first.ins, sync=True, reason="defer past first matmul"
                )
            for d in x1_dmas:
                add_dep_helper(
                    d.ins, mm_oc1_first.ins, sync=True, reason="defer image-1 load"
                )
    nc.compile()
    return nc


def _get_nc(mode):
    if mode not in _CACHE:
        _CACHE[mode] = _build_conv(mode)
    return _CACHE[mode]


# test-harness hooks: set TRACE=True before calling kernel() to capture an
# NTFF profile; LAST_RESULTS then holds the BassKernelResults.
TRACE = False
LAST_RESULTS = None


def kernel(x, weight, bias):
    global LAST_RESULTS
    mode = MODE
    x = np.ascontiguousarray(np.asarray(x), dtype=np.float32)
    w = np.ascontiguousarray(np.asarray(weight), dtype=np.float32)
    b = np.ascontiguousarray(np.asarray(bias), dtype=np.float32)
    xp = np.pad(x, ((0, 0), (0, 0), (1, 1), (1, 1)))
    # wt[i, (kh kw o)] = w[o, i, kh, kw]
    wt = np.ascontiguousarray(w.transpose(1, 2, 3, 0).reshape(CIN, KH * KW * COUT))

    if mode in ("fp32", "fp32r"):
        per_core = [
            {"x": xp[c * NPER : (c + 1) * NPER], "wt": wt, "bias": b}
            for c in range(NCORES)
        ]
    else:
        import ml_dtypes

        bfl = ml_dtypes.bfloat16
        if mode == "bf16":
            xh = xp.astype(bfl)
            wth = wt.astype(bfl)
            per_core = [
                {"x": xh[c * NPER : (c + 1) * NPER], "wt": wth, "bias": b}
                for c in range(NCORES)
            ]
        else:  # split3
            xh = xp.astype(bfl)
            xl = (xp - xh.astype(np.float32)).astype(bfl)
            wh = wt.astype(bfl)
            wl = (wt - wh.astype(np.float32)).astype(bfl)
            per_core = [
                {
                    "xh": xh[c * NPER : (c + 1) * NPER],
                    "xl": xl[c * NPER : (c + 1) * NPER],
                    "wh": wh,
                    "wl": wl,
                    "bias": b,
                }
                for c in range(NCORES)
            ]

    kwargs = {}
    if TRACE:
        kwargs = dict(trace=True, trace_cores=[0])
    res = run_bass_kernel_spmd(
        _get_nc(mode), per_core, core_ids=list(range(NCORES)), **kwargs
    )
    LAST_RESULTS = res
    return np.concatenate([r["out"] for r in res.results], axis=0)


# revision 26
# speedup vs baseline: 1.0578x; 1.0218x over previous
"""Conv2D 3x3 stride-1 pad-1 (NCHW) as implicit GEMM on 8 NeuronCores.

Strategy: data-parallel over batch (32 imgs -> 4 per core). The input is
zero-padded on the host to (*, 128, 58, 58) so each image DMAs contiguously
into an SBUF tile [C=128, 58, 58] with input channels on partitions.
Weights are preprocessed host-side to [I=128, (kh kw o)] so each
(tap, ochunk) slice is a ready [K=128, M=128] stationary operand.
Output rows are processed in groups of 8 (moving free dim N = 8*56 = 448),
accumulating the 9 filter taps into one PSUM bank per row-group.

x (4,128,58,58) -> out (4,256,56,56) f32 per core; no collectives.
"""

import os
import sys

import numpy as np

if "/opt/trn_rl_repo" not in sys.path:
    sys.path.insert(0, "/opt/trn_rl_repo")

from concourse import bacc, bass, mybir  # noqa: E402
from concourse.bass_utils import run_bass_kernel_spmd  # noqa: E402
from concourse.tile import TileContext, add_dep_helper  # noqa: E402

N_FULL, CIN, H, W = 32, 128, 56, 56
COUT = 256
KH = KW = 3
NCORES = 8
NPER = N_FULL // NCORES  # 4 images per core
HP, WP = H + 2, W + 2  # 58 x 58 padded
ROWS = 8  # output rows per matmul group
NFREE = ROWS * W  # 448 moving free dim (<= 512 for 4-byte dtypes)
NGROUPS = H // ROWS  # 7
OCH = COUT // 128  # 2 output-channel chunks

# fp32r: full fp32 data streamed through the PE at bf16 rate (free dim >= 256).
MODE = os.environ.get("CONV_MM_MODE", "fp32r")

_CACHE = {}


def _build_conv(mode):
    f32 = mybir.dt.float32
    bf16 = mybir.dt.bfloat16
    if mode == "fp32":
        mm_dt, io_dt = f32, f32
    elif mode == "fp32r":
        mm_dt, io_dt = mybir.dt.float32r, f32
    elif mode in ("bf16", "split3"):
        mm_dt, io_dt = bf16, bf16
    else:
        raise ValueError(mode)

    # Bacc (not raw Bass): its compile pipeline legalizes sync waits --
    # TRN2 instructions carry at most one wait slot.
    nc = bacc.Bacc(None, target_bir_lowering=False)

    if mode == "split3":
        x_names = ["xh", "xl"]
        w_names = ["wh", "wl"]
        # (x_idx, w_idx) matmul passes: hh + hl + lh ~ full fp32 product
        terms = [(0, 0), (0, 1), (1, 0)]
    else:
        x_names = ["x"]
        w_names = ["wt"]
        terms = [(0, 0)]

    x_par = [
        nc.declare_dram_parameter(nm, [NPER, CIN, HP, WP], io_dt, isOutput=False)
        for nm in x_names
    ]
    w_par = [
        nc.declare_dram_parameter(nm, [CIN, KH * KW * COUT], io_dt, isOutput=False)
        for nm in w_names
    ]
    bias_par = nc.declare_dram_parameter("bias", [COUT], f32, isOutput=False)
    out_par = nc.declare_dram_parameter("out", [NPER, COUT, H, W], f32, isOutput=True)
    out_flat = out_par.rearrange("n o h w -> n o (h w)")

    def mmview(ap):
        return ap.bitcast(mm_dt) if mm_dt != io_dt else ap

    nmm_per_psum = KH * KW * len(terms)

    with TileContext(nc) as tc:
        with (
            tc.tile_pool(name="const", bufs=1) as cpool,
            tc.tile_pool(name="xpad", bufs=1) as xpool,
            tc.tile_pool(name="psum", bufs=8, space="PSUM") as ppool,
            tc.tile_pool(name="outp", bufs=4) as opool,
        ):
            # Weights arrive in two chunks: the oc0 halves of all taps are on
            # the critical path to the first matmul; oc1 halves are deferred
            # behind it so the first-image load gets full HBM bandwidth.
            w_sb = []
            tail_dmas = []  # released once the first matmul has issued
            for wi, wp in enumerate(w_par):
                t = cpool.tile([CIN, KH * KW * COUT], mm_dt, tag=f"w{wi}", name="w")
                t3 = t.rearrange("p (t o) -> p t o", t=KH * KW)
                w3 = mmview(wp[:]).rearrange("p (t o) -> p t o", t=KH * KW)
                # One dma_start tops out well below HBM bandwidth; split the
                # critical oc0 chunk across two queues.
                nc.sync.dma_start(out=t3[:, 0:5, 0:128], in_=w3[:, 0:5, 0:128])
                nc.sync.dma_start(out=t3[:, 5:9, 0:128], in_=w3[:, 5:9, 0:128])
                d = nc.sync.dma_start(out=t3[:, :, 128:256], in_=w3[:, :, 128:256])
                tail_dmas.append(d)
                w_sb.append(t)
            bias_sb = cpool.tile([128, OCH], f32, tag="bias")
            nc.sync.dma_start(
                out=bias_sb[:], in_=bias_par.rearrange("(a b) -> b a", b=128)
            )

            # Two padded-x buffers per input tensor (double buffering across
            # images); the zero borders come in with the host-padded DMA.
            xpads = []  # [buf][x_idx] -> tile
            for b in range(2):
                per_buf = []
                for xi in range(len(x_par)):
                    t = xpool.tile(
                        [CIN, HP, WP], mm_dt, tag=f"xpad{b}_{xi}", name="xpad"
                    )
                    per_buf.append(t)
                xpads.append(per_buf)

            XSPLIT = 34  # padded rows [0,34) cover row-groups 0-3
            mm_first = None
            mm_oc1_first = None
            x1_dmas = []  # image-1 loads, deferred until the oc1 pass starts
            for n in range(NPER):
                bufs = xpads[n % 2]
                for xi, xp in enumerate(x_par):
                    if n == 0:
                        # Chunk A (rows 0-33) unblocks groups 0-3; split it
                        # across three queues for bandwidth. Chunk B follows
                        # behind the first matmul.
                        for r0, r1 in ((0, 12), (12, 23), (23, XSPLIT)):
                            nc.sync.dma_start(
                                out=bufs[xi][:, r0:r1, :],
                                in_=mmview(xp[0])[:, r0:r1, :],
                            )
                        d = nc.sync.dma_start(
                            out=bufs[xi][:, XSPLIT:HP, :],
                            in_=mmview(xp[0])[:, XSPLIT:HP, :],
                        )
                        tail_dmas.append(d)
                    else:
                        # gpsimd queue: slot-reuse waits must not block the
                        # sync queue's output DMAs.
                        d = nc.gpsimd.dma_start(out=bufs[xi][:], in_=mmview(xp[n]))
                        if n == 1:
                            x1_dmas.append(d)
                for oc in range(OCH):
                    psums = [
                        ppool.tile([128, NFREE], f32, tag="ps", name="ps")
                        for _ in range(NGROUPS)
                    ]
                    i_mm = 0
                    for xi, wi in terms:
                        xt = bufs[xi]
                        for tap in range(KH * KW):
                            kh, kw = divmod(tap, KW)
                            lhsT = w_sb[wi][
                                :, tap * COUT + oc * 128 : tap * COUT + oc * 128 + 128
                            ]
                            for g in range(NGROUPS):
                                mm = nc.tensor.matmul(
                                    psums[g][:],
                                    lhsT,
                                    xt[
                                        :,
                                        g * ROWS + kh : g * ROWS + kh + ROWS,
                                        kw : kw + W,
                                    ],
                                    start=(i_mm == 0),
                                    stop=(i_mm == nmm_per_psum - 1),
                                )
                                if n == 0 and i_mm == 0 and g == 0:
                                    if oc == 0:
                                        mm_first = mm
                                    else:
                                        mm_oc1_first = mm
                            i_mm += 1
                    for g in range(NGROUPS):
                        ot = opool.tile([128, NFREE], f32, tag="ot", name="ot")
                        nc.vector.tensor_scalar_add(
                            out=ot[:], in0=psums[g][:], scalar1=bias_sb[:, oc : oc + 1]
                        )
                        nc.sync.dma_start(
                            out=out_flat[
                                n,
                                oc * 128 : (oc + 1) * 128,
                                g * NFREE : (g + 1) * NFREE,
                            ],
                            in_=ot[:],
                        )
            for d in tail_dmas:
                add_dep_helper(
                    d.ins, mm_first.ins, sync=True, reason="defer past first matmul"
                )
            for d in x1_dmas:
                add_dep_helper(
                    d.ins, mm_oc1_first.ins, sync=True, reason="defer image-1 load"
                )
    nc.compile()
    return nc


def _get_nc(mode):
    if mode not in _CACHE:
        _CACHE[mode] = _build_conv(mode)
    return _CACHE[mode]


# test-harness hooks: set TRACE=True before calling kernel() to capture an
# NTFF profile; LAST_RESULTS then holds the BassKernelResults.
TRACE = False
LAST_RESULTS = None


def kernel(x, weight, bias):
    global LAST_RESULTS
    mode = MODE
    x = np.ascontiguousarray(np.asarray(x), dtype=np.float32)
    w = np.ascontiguousarray(np.asarray(weight), dtype=np.float32)
    b = np.ascontiguousarray(np.asarray(bias), dtype=np.float32)
    xp = np.pad(x, ((0, 0), (0, 0), (1, 1), (1, 1)))
    # wt[i, (kh kw o)] = w[o, i, kh, kw]
    wt = np.ascontiguousarray(w.transpose(1, 2, 3, 0).reshape(CIN, KH * KW * COUT))

    if mode in ("fp32", "fp32r"):
        per_core = [
            {"x": xp[c * NPER : (c + 1) * NPER], "wt": wt, "bias": b}
            for c in range(NCORES)
        ]
    else:
        import ml_dtypes

        bfl = ml_dtypes.bfloat16
        if mode == "bf16":
            xh = xp.astype(bfl)
            wth = wt.astype(bfl)
            per_core = [
                {"x": xh[c * NPER : (c + 1) * NPER], "wt": wth, "bias": b}
                for c in range(NCORES)
            ]
        else:  # split3
            xh = xp.astype(bfl)
            xl = (xp - xh.astype(np.float32)).astype(bfl)
            wh = wt.astype(bfl)
            wl = (wt - wh.astype(np.float32)).astype(bfl)
            per_core = [
                {
                    "xh": xh[c * NPER : (c + 1) * NPER],
                    "xl": xl[c * NPER : (c + 1) * NPER],
                    "wh": wh,
                    "wl": wl,
                    "bias": b,
                }
                for c in range(NCORES)
            ]

    kwargs = {}
    if TRACE:
        kwargs = dict(trace=True, trace_cores=[0])
    res = run_bass_kernel_spmd(
        _get_nc(mode), per_core, core_ids=list(range(NCORES)), **kwargs
    )
    LAST_RESULTS = res
    return np.concatenate([r["out"] for r in res.results], axis=0)


# revision 29
# speedup vs baseline: 1.0601x; 1.0021x over previous
"""Conv2D 3x3 stride-1 pad-1 (NCHW) as implicit GEMM on 8 NeuronCores.

Strategy: data-parallel over batch (32 imgs -> 4 per core). The input is
zero-padded on the host to (*, 128, 58, 58) so each image DMAs contiguously
into an SBUF tile [C=128, 58, 58] with input channels on partitions.
Weights are preprocessed host-side to [I=128, (kh kw o)] so each
(tap, ochunk) slice is a ready [K=128, M=128] stationary operand.
Output rows are processed in groups of 8 (moving free dim N = 8*56 = 448),
accumulating the 9 filter taps into one PSUM bank per row-group.

x (4,128,58,58) -> out (4,256,56,56) f32 per core; no collectives.
"""

import os
import sys

import numpy as np

if "/opt/trn_rl_repo" not in sys.path:
    sys.path.insert(0, "/opt/trn_rl_repo")

from concourse import bacc, bass, mybir  # noqa: E402
from concourse.bass_utils import run_bass_kernel_spmd  # noqa: E402
from concourse.tile import TileContext, add_dep_helper  # noqa: E402

N_FULL, CIN, H, W = 32, 128, 56, 56
COUT = 256
KH = KW = 3
NCORES = 8
NPER = N_FULL // NCORES  # 4 images per core
HP, WP = H + 2, W + 2  # 58 x 58 padded
ROWS = 8  # output rows per matmul group
NFREE = ROWS * W  # 448 moving free dim (<= 512 for 4-byte dtypes)
NGROUPS = H // ROWS  # 7
OCH = COUT // 128  # 2 output-channel chunks

# fp32r: full fp32 data streamed through the PE at bf16 rate (free dim >= 256).
MODE = os.environ.get("CONV_MM_MODE", "fp32r")

_CACHE = {}


def _build_conv(mode):
    f32 = mybir.dt.float32
    bf16 = mybir.dt.bfloat16
    if mode == "fp32":
        mm_dt, io_dt = f32, f32
    elif mode == "fp32r":
        mm_dt, io_dt = mybir.dt.float32r, f32
    elif mode in ("bf16", "split3"):
        mm_dt, io_dt = bf16, bf16
    else:
        raise ValueError(mode)

    # Bacc (not raw Bass): its compile pipeline legalizes sync waits --
    # TRN2 instructions carry at most one wait slot.
    nc = bacc.Bacc(None, target_bir_lowering=False)

    if mode == "split3":
        x_names = ["xh", "xl"]
        w_names = ["wh", "wl"]
        # (x_idx, w_idx) matmul passes: hh + hl + lh ~ full fp32 product
        terms = [(0, 0), (0, 1), (1, 0)]
    else:
        x_names = ["x"]
        w_names = ["wt"]
        terms = [(0, 0)]

    x_par = [
        nc.declare_dram_parameter(nm, [NPER, CIN, HP, WP], io_dt, isOutput=False)
        for nm in x_names
    ]
    w_par = [
        nc.declare_dram_parameter(nm, [CIN, KH * KW * COUT], io_dt, isOutput=False)
        for nm in w_names
    ]
    bias_par = nc.declare_dram_parameter("bias", [COUT], f32, isOutput=False)
    out_par = nc.declare_dram_parameter("out", [NPER, COUT, H, W], f32, isOutput=True)
    out_flat = out_par.rearrange("n o h w -> n o (h w)")

    def mmview(ap):
        return ap.bitcast(mm_dt) if mm_dt != io_dt else ap

    nmm_per_psum = KH * KW * len(terms)

    with TileContext(nc) as tc:
        with (
            tc.tile_pool(name="const", bufs=1) as cpool,
            tc.tile_pool(name="xpad", bufs=1) as xpool,
            tc.tile_pool(name="psum", bufs=8, space="PSUM") as ppool,
            tc.tile_pool(name="outp", bufs=4) as opool,
        ):
            # Weights arrive in two chunks: the oc0 halves of all taps are on
            # the critical path to the first matmul; oc1 halves are deferred
            # behind it so the first-image load gets full HBM bandwidth.
            w_sb = []
            tail_dmas = []  # released once the first matmul has issued
            for wi, wp in enumerate(w_par):
                t = cpool.tile([CIN, KH * KW * COUT], mm_dt, tag=f"w{wi}", name="w")
                t3 = t.rearrange("p (t o) -> p t o", t=KH * KW)
                w3 = mmview(wp[:]).rearrange("p (t o) -> p t o", t=KH * KW)
                # One dma_start tops out well below HBM bandwidth; split the
                # critical oc0 chunk across two queues.
                nc.sync.dma_start(out=t3[:, 0:5, 0:128], in_=w3[:, 0:5, 0:128])
                nc.sync.dma_start(out=t3[:, 5:9, 0:128], in_=w3[:, 5:9, 0:128])
                d = nc.sync.dma_start(out=t3[:, :, 128:256], in_=w3[:, :, 128:256])
                tail_dmas.append(d)
                w_sb.append(t)
            bias_sb = cpool.tile([128, OCH], f32, tag="bias")
            nc.sync.dma_start(
                out=bias_sb[:], in_=bias_par.rearrange("(a b) -> b a", b=128)
            )

            # HAM pre-warm: dependency-free junk matmuls run during the
            # initial DMA wait so the PE clock gate is at 8/8 (2.4 GHz) when
            # the real stream starts. Reads an uninitialized tile nothing
            # else touches; results are never consumed.
            jnk = cpool.tile([128, 512], f32, tag="jnk")
            nc.vector.memset(jnk[:], 1.0)
            jnk_mm = jnk if mm_dt == f32 else jnk.bitcast(mm_dt)
            ps_jnk = ppool.tile([128, NFREE], f32, tag="ps", name="ps")
            for _ in range(8):
                nc.tensor.matmul(
                    ps_jnk[:],
                    jnk_mm[:, 0:128],
                    jnk_mm[:, 0:NFREE],
                    start=True,
                    stop=True,
                )

            # Two padded-x buffers per input tensor (double buffering across
            # images); the zero borders come in with the host-padded DMA.
            xpads = []  # [buf][x_idx] -> tile
            for b in range(2):
                per_buf = []
                for xi in range(len(x_par)):
                    t = xpool.tile(
                        [CIN, HP, WP], mm_dt, tag=f"xpad{b}_{xi}", name="xpad"
                    )
                    per_buf.append(t)
                xpads.append(per_buf)

            XSPLIT = 34  # padded rows [0,34) cover row-groups 0-3
            mm_first = None
            mm_oc1_first = None
            x1_dmas = []  # image-1 loads, deferred until the oc1 pass starts
            for n in range(NPER):
                bufs = xpads[n % 2]
                for xi, xp in enumerate(x_par):
                    if n == 0:
                        # Chunk A (rows 0-33) unblocks groups 0-3; split it
                        # across three queues for bandwidth. Chunk B follows
                        # behind the first matmul.
                        for r0, r1 in ((0, 12), (12, 23), (23, XSPLIT)):
                            nc.sync.dma_start(
                                out=bufs[xi][:, r0:r1, :],
                                in_=mmview(xp[0])[:, r0:r1, :],
                            )
                        d = nc.sync.dma_start(
                            out=bufs[xi][:, XSPLIT:HP, :],
                            in_=mmview(xp[0])[:, XSPLIT:HP, :],
                        )
                        tail_dmas.append(d)
                    else:
                        # gpsimd queue: slot-reuse waits must not block the
                        # sync queue's output DMAs.
                        d = nc.gpsimd.dma_start(out=bufs[xi][:], in_=mmview(xp[n]))
                        if n == 1:
                            x1_dmas.append(d)
                for oc in range(OCH):
                    psums = [
                        ppool.tile([128, NFREE], f32, tag="ps", name="ps")
                        for _ in range(NGROUPS)
                    ]
                    i_mm = 0
                    for xi, wi in terms:
                        xt = bufs[xi]
                        for tap in range(KH * KW):
                            kh, kw = divmod(tap, KW)
                            lhsT = w_sb[wi][
                                :, tap * COUT + oc * 128 : tap * COUT + oc * 128 + 128
                            ]
                            for g in range(NGROUPS):
                                mm = nc.tensor.matmul(
                                    psums[g][:],
                                    lhsT,
                                    xt[
                                        :,
                                        g * ROWS + kh : g * ROWS + kh + ROWS,
                                        kw : kw + W,
                                    ],
                                    start=(i_mm == 0),
                                    stop=(i_mm == nmm_per_psum - 1),
                                )
                                if n == 0 and i_mm == 0 and g == 0:
                                    if oc == 0:
                                        mm_first = mm
                                    else:
                                        mm_oc1_first = mm
                            i_mm += 1
                    for g in range(NGROUPS):
                        ot = opool.tile([128, NFREE], f32, tag="ot", name="ot")
                        nc.vector.tensor_scalar_add(
                            out=ot[:], in0=psums[g][:], scalar1=bias_sb[:, oc : oc + 1]
                        )
                        nc.sync.dma_start(
                            out=out_flat[
                                n,
                                oc * 128 : (oc + 1) * 128,
                                g * NFREE : (g + 1) * NFREE,
                            ],
                            in_=ot[:],
                        )
            for d in tail_dmas:
                add_dep_helper(
                    d.ins, mm_first.ins, sync=True, reason="defer past first matmul"
                )
            for d in x1_dmas:
                add_dep_helper(
                    d.ins, mm_oc1_first.ins, sync=True, reason="defer image-1 load"
                )
    nc.compile()
    return nc


def _get_nc(mode):
    if mode not in _CACHE:
        _CACHE[mode] = _build_conv(mode)
    return _CACHE[mode]


# test-harness hooks: set TRACE=True before calling kernel() to capture an
# NTFF profile; LAST_RESULTS then holds the BassKernelResults.
TRACE = False
LAST_RESULTS = None


def kernel(x, weight, bias):
    global LAST_RESULTS
    mode = MODE
    x = np.ascontiguousarray(np.asarray(x), dtype=np.float32)
    w = np.ascontiguousarray(np.asarray(weight), dtype=np.float32)
    b = np.ascontiguousarray(np.asarray(bias), dtype=np.float32)
    xp = np.pad(x, ((0, 0), (0, 0), (1, 1), (1, 1)))
    # wt[i, (kh kw o)] = w[o, i, kh, kw]
    wt = np.ascontiguousarray(w.transpose(1, 2, 3, 0).reshape(CIN, KH * KW * COUT))

    if mode in ("fp32", "fp32r"):
        per_core = [
            {"x": xp[c * NPER : (c + 1) * NPER], "wt": wt, "bias": b}
            for c in range(NCORES)
        ]
    else:
        import ml_dtypes

        bfl = ml_dtypes.bfloat16
        if mode == "bf16":
            xh = xp.astype(bfl)
            wth = wt.astype(bfl)
            per_core = [
                {"x": xh[c * NPER : (c + 1) * NPER], "wt": wth, "bias": b}
                for c in range(NCORES)
            ]
        else:  # split3
            xh = xp.astype(bfl)
            xl = (xp - xh.astype(np.float32)).astype(bfl)
            wh = wt.astype(bfl)
            wl = (wt - wh.astype(np.float32)).astype(bfl)
            per_core = [
                {
                    "xh": xh[c * NPER : (c + 1) * NPER],
                    "xl": xl[c * NPER : (c + 1) * NPER],
                    "wh": wh,
                    "wl": wl,
                    "bias": b,
                }
                for c in range(NCORES)
            ]

    kwargs = {}
    if TRACE:
        kwargs = dict(trace=True, trace_cores=[0])
    res = run_bass_kernel_spmd(
        _get_nc(mode), per_core, core_ids=list(range(NCORES)), **kwargs
    )
    LAST_RESULTS = res
    return np.concatenate([r["out"] for r in res.results], axis=0)
